# revision 16
# baseline (speedup 1.0000x reference)
"""Deformable self-attention TRN2 kernel.

Sharding: 8 cores = batch(4) x head-group(2).  Each core handles one batch
element and 4 heads (128 of 256 v/out channels), producing a partial
out-projection; the host sums the two partials per batch and adds out_b.

Per-core pipeline (Tile-scheduled):
  1. x[b] -> SBUF, cast to bf16 zero-padded [128, 2cc, 98*98]
  2. conv3x3 (48 ch: px/py/aw) + base-coord matmul + v-proj, all bf16 on PE
  3. weight chain (DVE/ACT): floor/frac, DMA repack to slab layout, masks,
     bilinear factors, softmax(aw), corner weights (bf16 -> DRAM) and
     gather indices (int16, wrapped layout for dma_gather)
  4. per head: quad-token array (4 shifted copies + XBAR transpose);
     per (head, point, img-half): SBUF-source dma_gather (channel-major
     quads).  The Pool engine runs ONLY the 32 gathers (descriptor-gen
     bound, ~35us each); every other DMA rides HWDGE (sync/scalar).
  5. expand corner weights across 32-ch partition groups (one 3D-AP DMA
     per (h,p,half) on HWDGE), DVE multiply, PE corner-reduction (M=32
     matmul) -> sampled, out-proj.

Engine-op SBUF APs must start at partition 0/32/64/96 (HW quadrant rule);
DMAs may use arbitrary partition ranges - repacks/broadcasts ride on DMA.
"""

import numpy as np
import ml_dtypes

H = 96
W = 96
HW = H * W          # 9216
NH = 8
NP = 4
DIM = 256
HD = 32
NCORES = 8
NHL = 4             # heads per core
PAD = 98
NPIX_PAD = PAD * PAD
CH = 2304           # chain pixel quarter
NQ = 4
NC384 = 24
TOK = 9344          # token array cols (73*128)
TOKR = 73
VPADL = 97
VBUF = 9472
IDXMAX = 9312.0
HALF = HW // 2      # 4608
NCH = 12

_CACHE: dict = {}


def _build_nc():
    import concourse.bacc as bacc
    import concourse.bass as bass
    import concourse.mybir as mybir
    import concourse.tile as tile

    f32 = mybir.dt.float32
    bf16 = mybir.dt.bfloat16
    i16 = mybir.dt.int16

    nc = bacc.Bacc("TRN2", target_bir_lowering=False, debug=False,
                   enable_asserts=False, num_devices=NCORES)

    x_d = nc.dram_tensor("x", [DIM, HW], f32, kind="ExternalInput").ap()
    convw_d = nc.dram_tensor("convw", [128, 2, 9, 48], bf16,
                             kind="ExternalInput").ap()
    cbias_d = nc.dram_tensor("cbias", [48, 1], f32, kind="ExternalInput").ap()
    basew_d = nc.dram_tensor("basew", [2, 48], bf16, kind="ExternalInput").ap()
    basein_d = nc.dram_tensor("basein", [2, HW], bf16,
                              kind="ExternalInput").ap()
    vw_d = nc.dram_tensor("vw", [128, 2, 128], bf16, kind="ExternalInput").ap()
    vb_d = nc.dram_tensor("vb", [128, 1], f32, kind="ExternalInput").ap()
    w2_d = nc.dram_tensor("w2", [128, 2, 128], bf16, kind="ExternalInput").ap()
    redsel_d = nc.dram_tensor("redsel", [128, 32], bf16,
                              kind="ExternalInput").ap()
    sumsel_d = nc.dram_tensor("sumsel", [64, 16], f32,
                              kind="ExternalInput").ap()
    bcastsel_d = nc.dram_tensor("bcastsel", [16, 64], f32,
                                kind="ExternalInput").ap()
    out_d = nc.dram_tensor("out", [DIM, HW], f32, kind="ExternalOutput").ap()

    with tile.TileContext(nc) as tc:
        with tc.tile_pool(name="consts", bufs=1) as cpool:
            convw = cpool.tile([128, 2, 9, 48], bf16)
            nc.sync.dma_start(out=convw, in_=convw_d)
            cbias = cpool.tile([48, 1], f32)
            nc.sync.dma_start(out=cbias, in_=cbias_d)
            basew = cpool.tile([2, 48], bf16)
            nc.sync.dma_start(out=basew, in_=basew_d)
            vw = cpool.tile([128, 2, 128], bf16)
            nc.sync.dma_start(out=vw, in_=vw_d)
            vb = cpool.tile([128, 1], f32)
            nc.sync.dma_start(out=vb, in_=vb_d)
            w2 = cpool.tile([128, 2, 128], bf16)
            nc.sync.dma_start(out=w2, in_=w2_d)
            redsel = cpool.tile([128, 32], bf16)
            nc.sync.dma_start(out=redsel, in_=redsel_d)
            sumsel = cpool.tile([64, 16], f32)
            nc.sync.dma_start(out=sumsel, in_=sumsel_d)
            bcastsel = cpool.tile([16, 64], f32)
            nc.sync.dma_start(out=bcastsel, in_=bcastsel_d)

            with tc.tile_pool(name="persist", bufs=1) as pp, \
                 tc.tile_pool(name="dpool", bufs=1, space="DRAM") as dp:
                v_cm = pp.tile([128, VBUF], bf16)
                nc.vector.memset(v_cm, 0.0)
                W4flat = dp.tile([64, HW], bf16)
                idx_wrap = pp.tile([128, HW], i16)
                sampled = pp.tile([128, HW], bf16)

                with tc.tile_pool(name="pw", bufs=1) as pw:
                    # PXY packed: rows 32q..+16 = px(hp), +16..+32 = py(hp)
                    # AWp packed: rows 32q..+16 = aw(hp)
                    PXY = pw.tile([128, CH], f32)
                    AWp = pw.tile([128, CH], f32)
                    nc.vector.memset(AWp, 0.0)
                    _phase1(nc, tc, bass, mybir, x_d, basein_d, convw,
                            cbias, basew, vw, vb, v_cm, PXY, AWp)
                    _phase2_chain(nc, tc, bass, mybir, PXY, AWp, sumsel,
                                  bcastsel, W4flat, idx_wrap)
                _phase3_sample(nc, tc, bass, mybir, v_cm, W4flat,
                               idx_wrap, redsel, sampled, w2, out_d)

    nc.compile()
    return nc


def _phase1(nc, tc, bass, mybir, x_d, basein_d, convw, cbias, basew, vw, vb,
            v_cm, PXY, AWp):
    f32 = mybir.dt.float32
    bf16 = mybir.dt.bfloat16
    Act = mybir.ActivationFunctionType

    with tc.tile_pool(name="xpad", bufs=1) as xp, \
         tc.tile_pool(name="ps1", bufs=3, space="PSUM") as ps1, \
         tc.tile_pool(name="ps2", bufs=3, space="PSUM") as ps2:
        # bf16 image, zero-padded; cast rides the SWDGE DMA (Pool is idle
        # here - the gathers come much later).  Only the 1-px border needs
        # zeroing.
        x_b = xp.tile([128, 2, NPIX_PAD], bf16)
        nc.vector.memset(x_b, 0.0)
        for cc in range(2):
            dst = x_b[:, cc, :].rearrange("p (r c) -> p r c", c=PAD)
            nc.gpsimd.dma_start(
                out=dst[:, 1:97, 1:97],
                in_=x_d[cc * 128:(cc + 1) * 128, :].rearrange(
                    "p (r c) -> p r c", c=W))
        xv = [x_b[:, cc, :].rearrange("p (r c) -> p r c", c=PAD)
              for cc in range(2)]
        basein = xp.tile([2, HW], bf16)
        nc.sync.dma_start(out=basein, in_=basein_d)

        for n in range(NC384):
            ps = ps1.tile([48, 384], f32, tag="convps")
            first = True
            for cc in range(2):
                for t in range(9):
                    ky, kx = t // 3, t % 3
                    rhs = xv[cc][:, 4 * n + ky:4 * n + ky + 4, kx:kx + 96]
                    nc.tensor.matmul(
                        out=ps, lhsT=convw[:, cc, t, :], rhs=rhs,
                        start=first, stop=False)
                    first = False
            nc.tensor.matmul(out=ps, lhsT=basew,
                             rhs=basein[:, 384 * n:384 * (n + 1)],
                             start=False, stop=True)
            q, j6 = n // 6, n % 6
            sl = slice(384 * j6, 384 * (j6 + 1))
            nc.scalar.activation(out=PXY[32 * q:32 * q + 32, sl],
                                 in_=ps[0:32, :], func=Act.Identity,
                                 bias=cbias[0:32], scale=1.0)
            nc.scalar.activation(out=AWp[32 * q:32 * q + 16, sl],
                                 in_=ps[32:48, :], func=Act.Identity,
                                 bias=cbias[32:48], scale=1.0)

            psv = ps2.tile([128, 384], f32, tag="vps")
            for cc in range(2):
                rhsv = xv[cc][:, 4 * n + 1:4 * n + 5, 1:97]
                nc.tensor.matmul(out=psv, lhsT=vw[:, cc, :], rhs=rhsv,
                                 start=(cc == 0), stop=(cc == 1))
            nc.scalar.activation(
                out=v_cm[:, VPADL + 384 * n:VPADL + 384 * (n + 1)],
                in_=psv, func=Act.Identity, bias=vb, scale=1.0)


def _phase2_chain(nc, tc, bass, mybir, PXY, AWp, sumsel, bcastsel,
                  W4flat, idx_wrap):
    f32 = mybir.dt.float32
    bf16 = mybir.dt.bfloat16
    i16 = mybir.dt.int16
    i32 = mybir.dt.int32
    Alu = mybir.AluOpType
    Act = mybir.ActivationFunctionType

    with tc.tile_pool(name="chain", bufs=1) as chp, \
         tc.tile_pool(name="ps3", bufs=2, space="PSUM") as ps3, \
         tc.tile_pool(name="ps4", bufs=2, space="PSUM") as ps4:
        def t128(tag, dt=f32):
            return chp.tile([128, CH], dt, tag=tag, name=f"ch_{tag}")

        def t64(tag, dt=f32):
            return chp.tile([64, CH], dt, tag=tag, name=f"ch_{tag}")

        # floor/frac on packed layout
        r32t = t128("cA", i32)
        nc.vector.tensor_copy(out=r32t, in_=PXY)
        rf = t128("cB")
        nc.vector.tensor_copy(out=rf, in_=r32t)
        dg = t128("cC")
        nc.vector.tensor_tensor(out=dg, in0=rf, in1=PXY, op=Alu.is_gt)
        fl = t128("cD")
        nc.vector.tensor_tensor(out=fl, in0=rf, in1=dg, op=Alu.subtract)
        fr = t128("cE")
        nc.vector.tensor_tensor(out=fr, in0=PXY, in1=fl, op=Alu.subtract)

        # DMA repack packed -> slab: x rows 16q+hp, y rows 64+16q+hp
        fls = t128("cA")
        frs = t128("cB")
        for src, dst in ((fl, fls), (fr, frs)):
            for q in range(NQ):
                nc.sync.dma_start(out=dst[16 * q:16 * q + 16, :],
                                  in_=src[32 * q:32 * q + 16, :])
                nc.sync.dma_start(out=dst[64 + 16 * q:64 + 16 * q + 16, :],
                                  in_=src[32 * q + 16:32 * q + 32, :])
        aws = t64("cH")
        for q in range(NQ):
            nc.sync.dma_start(out=aws[16 * q:16 * q + 16, :],
                              in_=AWp[32 * q:32 * q + 16, :])

        # masks
        c0 = t128("cC")
        nc.vector.tensor_scalar(out=c0, in0=fls, scalar1=0.0, scalar2=95.0,
                                op0=Alu.max, op1=Alu.min)
        m0 = t128("cF")
        nc.vector.tensor_tensor(out=m0, in0=c0, in1=fls, op=Alu.is_equal)
        c1 = t128("cC")
        nc.vector.tensor_scalar(out=c1, in0=fls, scalar1=-1.0, scalar2=94.0,
                                op0=Alu.max, op1=Alu.min)
        m1 = t128("cG")
        nc.vector.tensor_tensor(out=m1, in0=c1, in1=fls, op=Alu.is_equal)

        omf = t128("cC")
        nc.vector.tensor_scalar(out=omf, in0=frs, scalar1=-1.0, scalar2=1.0,
                                op0=Alu.mult, op1=Alu.add)
        f0 = t128("cD")
        nc.vector.tensor_tensor(out=f0, in0=omf, in1=m0, op=Alu.mult)
        f1 = t128("cE")
        nc.vector.tensor_tensor(out=f1, in0=frs, in1=m1, op=Alu.mult)

        # softmax over points, folded into y-factors
        exps = t64("cI")
        nc.scalar.activation(out=exps, in_=aws, func=Act.Exp, scale=1.0)
        rsum = chp.tile([16, CH], f32, tag="cI2")
        awn = t64("cH")
        for j in range(6):
            sl = slice(384 * j, 384 * (j + 1))
            pss = ps3.tile([16, 384], f32, tag="ssum")
            nc.tensor.matmul(out=pss, lhsT=sumsel, rhs=exps[:, sl],
                             start=True, stop=True)
            nc.vector.reciprocal_approx_fast(out=rsum[:, sl], in_=pss)
            psb = ps4.tile([64, 384], f32, tag="sbc")
            nc.tensor.matmul(out=psb, lhsT=bcastsel, rhs=rsum[:, sl],
                             start=True, stop=True)
            nc.vector.tensor_tensor(out=awn[:, sl], in0=exps[:, sl],
                                    in1=psb, op=Alu.mult)

        # TT/TS operands must share partitions: stage y-halves down to 0:64
        fy0c = t64("cK")
        nc.scalar.activation(out=fy0c, in_=f0[64:128, :], func=Act.Copy,
                             scale=1.0)
        f0y = t64("cF")
        nc.vector.tensor_tensor(out=f0y, in0=fy0c, in1=awn, op=Alu.mult)
        fy1c = t64("cK")
        nc.scalar.activation(out=fy1c, in_=f1[64:128, :], func=Act.Copy,
                             scale=1.0)
        f1y = t64("cG")
        nc.vector.tensor_tensor(out=f1y, in0=fy1c, in1=awn, op=Alu.mult)

        w4q = [chp.tile([64, CH], bf16, tag=f"cw{i}", name=f"w4q{i}")
               for i in range(4)]
        nc.vector.tensor_tensor(out=w4q[0], in0=f0[0:64, :], in1=f0y,
                                op=Alu.mult)
        nc.vector.tensor_tensor(out=w4q[1], in0=f1[0:64, :], in1=f0y,
                                op=Alu.mult)
        nc.vector.tensor_tensor(out=w4q[2], in0=f0[0:64, :], in1=f1y,
                                op=Alu.mult)
        nc.vector.tensor_tensor(out=w4q[3], in0=f1[0:64, :], in1=f1y,
                                op=Alu.mult)

        # gather index: t = clip(y0*96 + x0 + 97, 0, 9312)
        flyc = t64("cK")
        nc.scalar.activation(out=flyc, in_=fls[64:128, :], func=Act.Copy,
                             scale=1.0)
        t1 = t64("cC")
        nc.vector.tensor_scalar(out=t1, in0=flyc,
                                scalar1=96.0, scalar2=97.0,
                                op0=Alu.mult, op1=Alu.add)
        t2 = t64("cI2")
        nc.vector.tensor_tensor(out=t2, in0=t1, in1=fls[0:64, :], op=Alu.add)
        t3 = t64("cC")
        nc.vector.tensor_scalar(out=t3, in0=t2, scalar1=0.0, scalar2=IDXMAX,
                                op0=Alu.max, op1=Alu.min)

        # int16 cast with within-row wrap permute: out[144a + b] = in[a + 16b]
        idx16 = chp.tile([64, CH], i16, tag="cidx")
        t3a, idx16a = t3[:], idx16[:]
        in_ap = bass.AP(tensor=t3a.tensor, offset=t3a.offset,
                        ap=[t3a.ap[0], [1, 16], [16, 144]])
        out_ap = bass.AP(tensor=idx16a.tensor, offset=idx16a.offset,
                         ap=[idx16a.ap[0], [144, 16], [1, 144]])
        nc.scalar.activation(out=out_ap, in_=in_ap, func=Act.Copy, scale=1.0)

        for c in range(4):
            for q in range(NQ):
                nc.sync.dma_start(
                    out=W4flat[16 * c:16 * c + 16, CH * q:CH * (q + 1)],
                    in_=w4q[c][16 * q:16 * q + 16, :])

        for hp in range(16):
            for q in range(NQ):
                row = idx16[16 * q + hp:16 * q + hp + 1, :]
                src = bass.AP(tensor=row.tensor, offset=row.offset,
                              ap=[row.ap[0], [144, 16], [1, 144]])
                nc.sync.dma_start(
                    out=idx_wrap[0:16,
                                 576 * hp + 144 * q:576 * hp + 144 * (q + 1)],
                    in_=src)
        for rr in range(1, 8):
            nc.sync.dma_start(out=idx_wrap[16 * rr:16 * rr + 16, :],
                              in_=idx_wrap[0:16, :])


def _phase3_sample(nc, tc, bass, mybir, v_cm, W4flat, idx_wrap, redsel,
                   sampled, w2, out_d):
    f32 = mybir.dt.float32
    bf16 = mybir.dt.bfloat16
    Alu = mybir.AluOpType
    Act = mybir.ActivationFunctionType

    with tc.tile_pool(name="samp", bufs=1) as sp, \
         tc.tile_pool(name="gpool", bufs=2) as gp, \
         tc.tile_pool(name="wpool", bufs=1) as wp, \
         tc.tile_pool(name="opool", bufs=2) as op, \
         tc.tile_pool(name="psr", bufs=6, space="PSUM") as psr, \
         tc.tile_pool(name="pso", bufs=2, space="PSUM") as pso:
        for h in range(NHL):
            v4cm = sp.tile([128, TOK], bf16, tag="v4cm")
            for c, dlt in enumerate((0, 1, 96, 97)):
                if c % 2 == 0:
                    nc.vector.tensor_copy(
                        out=v4cm[32 * c:32 * c + 32, :],
                        in_=v_cm[32 * h:32 * h + 32, dlt:dlt + TOK])
                else:
                    nc.scalar.activation(
                        out=v4cm[32 * c:32 * c + 32, :],
                        in_=v_cm[32 * h:32 * h + 32, dlt:dlt + TOK],
                        func=Act.Copy, scale=1.0)
            tokens = sp.tile([128, TOKR, 128], bf16, tag="tokens")
            nc.sync.dma_start_transpose(out=tokens[:], in_=v4cm[:])

            for half in range(2):
                gt = [gp.tile([128, 1, HALF], bf16, tag=f"g{p % 3}",
                              name=f"gt{p}") for p in range(NP)]
                wexp = [wp.tile([128, HALF], bf16, tag=f"w{p % 2}",
                                name=f"wexp{p}") for p in range(NP)]
                for p in range(NP):
                    hp = h * 4 + p
                    # one 3D-AP broadcast DMA: rows {16c+hp} x32 replication
                    row0 = W4flat[hp:hp + 1, HALF * half:HALF * (half + 1)]
                    src = bass.AP(tensor=row0.tensor, offset=row0.offset,
                                  ap=[[0, 1], [16 * HW, 4], [0, 32],
                                      [1, HALF]])
                    nc.scalar.dma_start(
                        out=wexp[p][:].rearrange("p (a b) -> p a b", a=1),
                        in_=src)
                    nc.gpsimd.dma_gather(
                        gt[p][:], tokens[:],
                        idx_wrap[:, 576 * hp + 288 * half:
                                 576 * hp + 288 * (half + 1)],
                        HALF, HALF, 128, transpose=True,
                        sbuf_tokens_per_rank=128,
                        sbuf_free_dim_per_rank=256,
                        single_packet=False)
                for p in range(NP):
                    nc.vector.tensor_tensor(out=gt[p][:, 0, :],
                                            in0=gt[p][:, 0, :], in1=wexp[p],
                                            op=Alu.mult)
                for n in range(NCH):
                    ng = NCH * half + n
                    bank = psr.tile([32, 384], f32, tag="red")
                    for p in range(NP):
                        nc.tensor.matmul(
                            out=bank, lhsT=redsel,
                            rhs=gt[p][:, 0, 384 * n:384 * (n + 1)],
                            start=(p == 0), stop=(p == 3))
                    nc.scalar.activation(
                        out=sampled[32 * h:32 * h + 32,
                                    384 * ng:384 * (ng + 1)],
                        in_=bank, func=Act.Copy, scale=1.0)

        # out-projection (overlaps the gather tail via Tile deps)
        for n in range(NC384):
            sl = slice(384 * n, 384 * (n + 1))
            for half in range(2):
                ob = pso.tile([128, 384], f32, tag="ob")
                nc.tensor.matmul(out=ob, lhsT=w2[:, half, :],
                                 rhs=sampled[:, sl], start=True, stop=True)
                osb = op.tile([128, 384], f32, tag="osb")
                nc.vector.tensor_copy(out=osb, in_=ob)
                nc.sync.dma_start(
                    out=out_d[half * 128:(half + 1) * 128, sl], in_=osb)


def _host_inputs(inputs):
    x = np.asarray(inputs["x"], dtype=np.float32)
    kv_w = np.asarray(inputs["kv_w"], dtype=np.float32)
    kv_b = np.asarray(inputs["kv_b"], dtype=np.float32)
    off_w = np.asarray(inputs["off_w"], dtype=np.float32)
    off_b = np.asarray(inputs["off_b"], dtype=np.float32)
    aw_w = np.asarray(inputs["aw_w"], dtype=np.float32)
    aw_b = np.asarray(inputs["aw_b"], dtype=np.float32)
    out_w = np.asarray(inputs["out_w"], dtype=np.float32)

    sx = (W - 1.0) / W
    sy = (H - 1.0) / H

    redsel = np.zeros((128, 32), np.float32)
    for c in range(4):
        redsel[32 * c + np.arange(32), np.arange(32)] = 1.0
    sumsel = np.zeros((64, 16), np.float32)
    bcastsel = np.zeros((16, 64), np.float32)
    for q in range(4):
        for hh in range(4):
            for p in range(4):
                sumsel[16 * q + 4 * hh + p, 4 * q + hh] = 1.0
                bcastsel[4 * q + hh, 16 * q + 4 * hh + p] = 1.0

    basein = np.zeros((2, HW), np.float32)
    basein[0] = np.arange(HW) % W
    basein[1] = np.arange(HW) // W
    basew = np.zeros((2, 48), np.float32)
    basew[0, 0:16] = 1.0
    basew[1, 16:32] = 1.0

    bf = ml_dtypes.bfloat16
    in_maps = []
    for core in range(NCORES):
        b, hg = core // 2, core % 2
        heads = list(range(4 * hg, 4 * hg + 4))

        convw = np.zeros((128, 2, 9, 48), np.float32)
        cbias = np.zeros((48, 1), np.float32)
        for j, gh in enumerate(heads):
            for p in range(NP):
                hp = j * 4 + p
                wx = off_w[gh * 8 + p * 2 + 0] * sx
                wy = off_w[gh * 8 + p * 2 + 1] * sy
                wa = aw_w[gh * 4 + p]
                for t in range(9):
                    ky, kx = t // 3, t % 3
                    for cc in range(2):
                        csl = slice(cc * 128, (cc + 1) * 128)
                        convw[:, cc, t, hp] = wx[csl, ky, kx]
                        convw[:, cc, t, 16 + hp] = wy[csl, ky, kx]
                        convw[:, cc, t, 32 + hp] = wa[csl, ky, kx]
                cbias[hp, 0] = off_b[gh * 8 + p * 2 + 0] * sx
                cbias[16 + hp, 0] = off_b[gh * 8 + p * 2 + 1] * sy
                cbias[32 + hp, 0] = aw_b[gh * 4 + p]

        vw = np.zeros((128, 2, 128), np.float32)
        vrows = kv_w[DIM + hg * 128:DIM + (hg + 1) * 128, :]
        for cc in range(2):
            vw[:, cc, :] = vrows[:, cc * 128:(cc + 1) * 128].T
        vb = kv_b[DIM + hg * 128:DIM + (hg + 1) * 128].reshape(128, 1)

        w2 = np.zeros((128, 2, 128), np.float32)
        for halfi in range(2):
            w2[:, halfi, :] = out_w[halfi * 128:(halfi + 1) * 128,
                                    hg * 128:(hg + 1) * 128].T

        in_maps.append({
            "x": np.ascontiguousarray(x[b]),
            "convw": convw.astype(bf),
            "cbias": cbias,
            "basew": basew.astype(bf),
            "basein": basein.astype(bf),
            "vw": vw.astype(bf),
            "vb": np.ascontiguousarray(vb),
            "w2": w2.astype(bf),
            "redsel": redsel.astype(bf),
            "sumsel": sumsel,
            "bcastsel": bcastsel,
        })
    return in_maps


def kernel(**inputs):
    from concourse import bass_utils

    if "nc" not in _CACHE:
        _CACHE["nc"] = _build_nc()
    nc = _CACHE["nc"]

    in_maps = _host_inputs(inputs)
    res = bass_utils.run_bass_kernel_spmd(nc, in_maps,
                                          core_ids=list(range(NCORES)))
    out_b = np.asarray(inputs["out_b"], dtype=np.float32)
    out = np.zeros((4, DIM, HW), np.float32)
    for b in range(4):
        out[b] = (res.results[2 * b]["out"] + res.results[2 * b + 1]["out"]
                  + out_b[:, None])
    return out


# revision 17
# speedup vs baseline: 1.1787x; 1.1787x over previous
"""Deformable self-attention TRN2 kernel.

Sharding: 8 cores = batch(4) x head-group(2).  Each core handles one batch
element and 4 heads (128 of 256 v/out channels), producing a partial
out-projection; the host sums the two partials per batch and adds out_b.

Per-core pipeline (Tile-scheduled):
  1. x[b] -> SBUF, cast to bf16 zero-padded [128, 2cc, 98*98]
  2. conv3x3 (48 ch: px/py/aw) + base-coord matmul + v-proj, all bf16 on PE
  3. weight chain (DVE/ACT): floor/frac, DMA repack to slab layout, masks,
     bilinear factors, softmax(aw), corner weights (bf16 -> DRAM) and
     gather indices (int16, wrapped layout for dma_gather)
  4. per head: quad-token array (4 shifted copies + XBAR transpose);
     per (head, point, img-half): SBUF-source dma_gather (channel-major
     quads).  The Pool engine runs ONLY the 32 gathers (descriptor-gen
     bound, ~35us each); every other DMA rides HWDGE (sync/scalar).
  5. expand corner weights across 32-ch partition groups (one 3D-AP DMA
     per (h,p,half) on HWDGE), DVE multiply, PE corner-reduction (M=32
     matmul) -> sampled, out-proj.

Engine-op SBUF APs must start at partition 0/32/64/96 (HW quadrant rule);
DMAs may use arbitrary partition ranges - repacks/broadcasts ride on DMA.
"""

import numpy as np
import ml_dtypes

H = 96
W = 96
HW = H * W          # 9216
NH = 8
NP = 4
DIM = 256
HD = 32
NCORES = 8
NHL = 4             # heads per core
PAD = 98
NPIX_PAD = PAD * PAD
CH = 2304           # chain pixel quarter
NQ = 4
NC384 = 24
TOK = 9344          # token array cols (73*128)
TOKR = 73
VPADL = 97
VBUF = 9472
IDXMAX = 9312.0
HALF = HW // 2      # 4608
NCH = 12

_CACHE: dict = {}


def _build_nc():
    import concourse.bacc as bacc
    import concourse.bass as bass
    import concourse.mybir as mybir
    import concourse.tile as tile

    f32 = mybir.dt.float32
    bf16 = mybir.dt.bfloat16
    i16 = mybir.dt.int16

    nc = bacc.Bacc("TRN2", target_bir_lowering=False, debug=False,
                   enable_asserts=False, num_devices=NCORES)

    x_d = nc.dram_tensor("x", [DIM, HW], f32, kind="ExternalInput").ap()
    convw_d = nc.dram_tensor("convw", [128, 2, 9, 48], bf16,
                             kind="ExternalInput").ap()
    cbias_d = nc.dram_tensor("cbias", [48, 1], f32, kind="ExternalInput").ap()
    basew_d = nc.dram_tensor("basew", [2, 48], bf16, kind="ExternalInput").ap()
    basein_d = nc.dram_tensor("basein", [2, HW], bf16,
                              kind="ExternalInput").ap()
    vw_d = nc.dram_tensor("vw", [128, 2, 128], bf16, kind="ExternalInput").ap()
    vb_d = nc.dram_tensor("vb", [128, 1], f32, kind="ExternalInput").ap()
    w2_d = nc.dram_tensor("w2", [128, 2, 128], bf16, kind="ExternalInput").ap()
    redsel_d = nc.dram_tensor("redsel", [128, 32], bf16,
                              kind="ExternalInput").ap()
    sumsel_d = nc.dram_tensor("sumsel", [64, 16], f32,
                              kind="ExternalInput").ap()
    bcastsel_d = nc.dram_tensor("bcastsel", [16, 64], f32,
                                kind="ExternalInput").ap()
    out_d = nc.dram_tensor("out", [DIM, HW], f32, kind="ExternalOutput").ap()

    with tile.TileContext(nc) as tc:
        with tc.tile_pool(name="consts", bufs=1) as cpool:
            convw = cpool.tile([128, 2, 9, 48], bf16)
            nc.sync.dma_start(out=convw, in_=convw_d)
            cbias = cpool.tile([48, 1], f32)
            nc.sync.dma_start(out=cbias, in_=cbias_d)
            basew = cpool.tile([2, 48], bf16)
            nc.sync.dma_start(out=basew, in_=basew_d)
            vw = cpool.tile([128, 2, 128], bf16)
            nc.sync.dma_start(out=vw, in_=vw_d)
            vb = cpool.tile([128, 1], f32)
            nc.sync.dma_start(out=vb, in_=vb_d)
            w2 = cpool.tile([128, 2, 128], bf16)
            nc.sync.dma_start(out=w2, in_=w2_d)
            redsel = cpool.tile([128, 32], bf16)
            nc.sync.dma_start(out=redsel, in_=redsel_d)
            sumsel = cpool.tile([64, 16], f32)
            nc.sync.dma_start(out=sumsel, in_=sumsel_d)
            bcastsel = cpool.tile([16, 64], f32)
            nc.sync.dma_start(out=bcastsel, in_=bcastsel_d)

            with tc.tile_pool(name="persist", bufs=1) as pp, \
                 tc.tile_pool(name="dpool", bufs=1, space="DRAM") as dp:
                v_cm = pp.tile([128, VBUF], bf16)
                nc.vector.memset(v_cm, 0.0)
                W4flat = dp.tile([64, HW], bf16)
                idx_wrap = pp.tile([128, HW], i16)
                sampled = pp.tile([128, HW], bf16)

                with tc.tile_pool(name="pw", bufs=1) as pw:
                    # PXY packed: rows 32q..+16 = px(hp), +16..+32 = py(hp)
                    # AWp packed: rows 32q..+16 = aw(hp)
                    PXY = pw.tile([128, CH], f32)
                    AWp = pw.tile([128, CH], f32)
                    nc.vector.memset(AWp, 0.0)
                    _phase1(nc, tc, bass, mybir, x_d, basein_d, convw,
                            cbias, basew, vw, vb, v_cm, PXY, AWp)
                    _phase2_chain(nc, tc, bass, mybir, PXY, AWp, sumsel,
                                  bcastsel, W4flat, idx_wrap)
                _phase3_sample(nc, tc, bass, mybir, v_cm, W4flat,
                               idx_wrap, redsel, sampled, w2, out_d)

    nc.compile()
    return nc


def _phase1(nc, tc, bass, mybir, x_d, basein_d, convw, cbias, basew, vw, vb,
            v_cm, PXY, AWp):
    f32 = mybir.dt.float32
    bf16 = mybir.dt.bfloat16
    Act = mybir.ActivationFunctionType

    with tc.tile_pool(name="xpad", bufs=1) as xp, \
         tc.tile_pool(name="ps1", bufs=3, space="PSUM") as ps1, \
         tc.tile_pool(name="ps2", bufs=3, space="PSUM") as ps2:
        # bf16 image, zero-padded; cast rides the SWDGE DMA (Pool is idle
        # here - the gathers come much later).  Only the 1-px border needs
        # zeroing.
        x_b = xp.tile([128, 2, NPIX_PAD], bf16)
        nc.vector.memset(x_b, 0.0)
        for cc in range(2):
            dst = x_b[:, cc, :].rearrange("p (r c) -> p r c", c=PAD)
            nc.gpsimd.dma_start(
                out=dst[:, 1:97, 1:97],
                in_=x_d[cc * 128:(cc + 1) * 128, :].rearrange(
                    "p (r c) -> p r c", c=W))
        xv = [x_b[:, cc, :].rearrange("p (r c) -> p r c", c=PAD)
              for cc in range(2)]
        basein = xp.tile([2, HW], bf16)
        nc.sync.dma_start(out=basein, in_=basein_d)

        for n in range(NC384):
            ps = ps1.tile([48, 384], f32, tag="convps")
            first = True
            for cc in range(2):
                for t in range(9):
                    ky, kx = t // 3, t % 3
                    rhs = xv[cc][:, 4 * n + ky:4 * n + ky + 4, kx:kx + 96]
                    nc.tensor.matmul(
                        out=ps, lhsT=convw[:, cc, t, :], rhs=rhs,
                        start=first, stop=False)
                    first = False
            nc.tensor.matmul(out=ps, lhsT=basew,
                             rhs=basein[:, 384 * n:384 * (n + 1)],
                             start=False, stop=True)
            q, j6 = n // 6, n % 6
            sl = slice(384 * j6, 384 * (j6 + 1))
            nc.scalar.activation(out=PXY[32 * q:32 * q + 32, sl],
                                 in_=ps[0:32, :], func=Act.Identity,
                                 bias=cbias[0:32], scale=1.0)
            nc.scalar.activation(out=AWp[32 * q:32 * q + 16, sl],
                                 in_=ps[32:48, :], func=Act.Identity,
                                 bias=cbias[32:48], scale=1.0)

            psv = ps2.tile([128, 384], f32, tag="vps")
            for cc in range(2):
                rhsv = xv[cc][:, 4 * n + 1:4 * n + 5, 1:97]
                nc.tensor.matmul(out=psv, lhsT=vw[:, cc, :], rhs=rhsv,
                                 start=(cc == 0), stop=(cc == 1))
            nc.scalar.activation(
                out=v_cm[:, VPADL + 384 * n:VPADL + 384 * (n + 1)],
                in_=psv, func=Act.Identity, bias=vb, scale=1.0)


def _phase2_chain(nc, tc, bass, mybir, PXY, AWp, sumsel, bcastsel,
                  W4flat, idx_wrap):
    f32 = mybir.dt.float32
    bf16 = mybir.dt.bfloat16
    i16 = mybir.dt.int16
    i32 = mybir.dt.int32
    Alu = mybir.AluOpType
    Act = mybir.ActivationFunctionType

    with tc.tile_pool(name="chain", bufs=1) as chp, \
         tc.tile_pool(name="ps3", bufs=2, space="PSUM") as ps3, \
         tc.tile_pool(name="ps4", bufs=2, space="PSUM") as ps4:
        def t128(tag, dt=f32):
            return chp.tile([128, CH], dt, tag=tag, name=f"ch_{tag}")

        def t64(tag, dt=f32):
            return chp.tile([64, CH], dt, tag=tag, name=f"ch_{tag}")

        # floor/frac on packed layout
        r32t = t128("cA", i32)
        nc.vector.tensor_copy(out=r32t, in_=PXY)
        rf = t128("cB")
        nc.vector.tensor_copy(out=rf, in_=r32t)
        dg = t128("cC")
        nc.vector.tensor_tensor(out=dg, in0=rf, in1=PXY, op=Alu.is_gt)
        fl = t128("cD")
        nc.vector.tensor_tensor(out=fl, in0=rf, in1=dg, op=Alu.subtract)
        fr = t128("cE")
        nc.vector.tensor_tensor(out=fr, in0=PXY, in1=fl, op=Alu.subtract)

        # DMA repack packed -> slab: x rows 16q+hp, y rows 64+16q+hp
        fls = t128("cA")
        frs = t128("cB")
        for src, dst in ((fl, fls), (fr, frs)):
            for q in range(NQ):
                nc.sync.dma_start(out=dst[16 * q:16 * q + 16, :],
                                  in_=src[32 * q:32 * q + 16, :])
                nc.sync.dma_start(out=dst[64 + 16 * q:64 + 16 * q + 16, :],
                                  in_=src[32 * q + 16:32 * q + 32, :])
        aws = t64("cH")
        for q in range(NQ):
            nc.sync.dma_start(out=aws[16 * q:16 * q + 16, :],
                              in_=AWp[32 * q:32 * q + 16, :])

        # masks
        c0 = t128("cC")
        nc.vector.tensor_scalar(out=c0, in0=fls, scalar1=0.0, scalar2=95.0,
                                op0=Alu.max, op1=Alu.min)
        m0 = t128("cF")
        nc.vector.tensor_tensor(out=m0, in0=c0, in1=fls, op=Alu.is_equal)
        c1 = t128("cC")
        nc.vector.tensor_scalar(out=c1, in0=fls, scalar1=-1.0, scalar2=94.0,
                                op0=Alu.max, op1=Alu.min)
        m1 = t128("cG")
        nc.vector.tensor_tensor(out=m1, in0=c1, in1=fls, op=Alu.is_equal)

        omf = t128("cC")
        nc.vector.tensor_scalar(out=omf, in0=frs, scalar1=-1.0, scalar2=1.0,
                                op0=Alu.mult, op1=Alu.add)
        f0 = t128("cD")
        nc.vector.tensor_tensor(out=f0, in0=omf, in1=m0, op=Alu.mult)
        f1 = t128("cE")
        nc.vector.tensor_tensor(out=f1, in0=frs, in1=m1, op=Alu.mult)

        # softmax over points, folded into y-factors
        exps = t64("cI")
        nc.scalar.activation(out=exps, in_=aws, func=Act.Exp, scale=1.0)
        rsum = chp.tile([16, CH], f32, tag="cI2")
        awn = t64("cH")
        for j in range(6):
            sl = slice(384 * j, 384 * (j + 1))
            pss = ps3.tile([16, 384], f32, tag="ssum")
            nc.tensor.matmul(out=pss, lhsT=sumsel, rhs=exps[:, sl],
                             start=True, stop=True)
            nc.vector.reciprocal_approx_fast(out=rsum[:, sl], in_=pss)
            psb = ps4.tile([64, 384], f32, tag="sbc")
            nc.tensor.matmul(out=psb, lhsT=bcastsel, rhs=rsum[:, sl],
                             start=True, stop=True)
            nc.vector.tensor_tensor(out=awn[:, sl], in0=exps[:, sl],
                                    in1=psb, op=Alu.mult)

        # TT/TS operands must share partitions: stage y-halves down to 0:64
        fy0c = t64("cK")
        nc.scalar.activation(out=fy0c, in_=f0[64:128, :], func=Act.Copy,
                             scale=1.0)
        f0y = t64("cF")
        nc.vector.tensor_tensor(out=f0y, in0=fy0c, in1=awn, op=Alu.mult)
        fy1c = t64("cK")
        nc.scalar.activation(out=fy1c, in_=f1[64:128, :], func=Act.Copy,
                             scale=1.0)
        f1y = t64("cG")
        nc.vector.tensor_tensor(out=f1y, in0=fy1c, in1=awn, op=Alu.mult)

        w4q = [chp.tile([64, CH], bf16, tag=f"cw{i}", name=f"w4q{i}")
               for i in range(4)]
        nc.vector.tensor_tensor(out=w4q[0], in0=f0[0:64, :], in1=f0y,
                                op=Alu.mult)
        nc.vector.tensor_tensor(out=w4q[1], in0=f1[0:64, :], in1=f0y,
                                op=Alu.mult)
        nc.vector.tensor_tensor(out=w4q[2], in0=f0[0:64, :], in1=f1y,
                                op=Alu.mult)
        nc.vector.tensor_tensor(out=w4q[3], in0=f1[0:64, :], in1=f1y,
                                op=Alu.mult)

        # gather index: t = clip(y0*96 + x0 + 97, 0, 9312)
        flyc = t64("cK")
        nc.scalar.activation(out=flyc, in_=fls[64:128, :], func=Act.Copy,
                             scale=1.0)
        t1 = t64("cC")
        nc.vector.tensor_scalar(out=t1, in0=flyc,
                                scalar1=96.0, scalar2=97.0,
                                op0=Alu.mult, op1=Alu.add)
        t2 = t64("cI2")
        nc.vector.tensor_tensor(out=t2, in0=t1, in1=fls[0:64, :], op=Alu.add)
        t3 = t64("cC")
        nc.vector.tensor_scalar(out=t3, in0=t2, scalar1=0.0, scalar2=IDXMAX,
                                op0=Alu.max, op1=Alu.min)

        # int16 cast with within-row wrap permute: out[144a + b] = in[a + 16b]
        idx16 = chp.tile([64, CH], i16, tag="cidx")
        t3a, idx16a = t3[:], idx16[:]
        in_ap = bass.AP(tensor=t3a.tensor, offset=t3a.offset,
                        ap=[t3a.ap[0], [1, 16], [16, 144]])
        out_ap = bass.AP(tensor=idx16a.tensor, offset=idx16a.offset,
                         ap=[idx16a.ap[0], [144, 16], [1, 144]])
        nc.scalar.activation(out=out_ap, in_=in_ap, func=Act.Copy, scale=1.0)

        for c in range(4):
            for q in range(NQ):
                nc.sync.dma_start(
                    out=W4flat[16 * c:16 * c + 16, CH * q:CH * (q + 1)],
                    in_=w4q[c][16 * q:16 * q + 16, :])

        for hp in range(16):
            for q in range(NQ):
                row = idx16[16 * q + hp:16 * q + hp + 1, :]
                src = bass.AP(tensor=row.tensor, offset=row.offset,
                              ap=[row.ap[0], [144, 16], [1, 144]])
                nc.sync.dma_start(
                    out=idx_wrap[0:16,
                                 576 * hp + 144 * q:576 * hp + 144 * (q + 1)],
                    in_=src)
        for rr in range(1, 8):
            nc.sync.dma_start(out=idx_wrap[16 * rr:16 * rr + 16, :],
                              in_=idx_wrap[0:16, :])


def _phase3_sample(nc, tc, bass, mybir, v_cm, W4flat, idx_wrap, redsel,
                   sampled, w2, out_d):
    f32 = mybir.dt.float32
    bf16 = mybir.dt.bfloat16
    Alu = mybir.AluOpType
    Act = mybir.ActivationFunctionType

    with tc.tile_pool(name="samp", bufs=1) as sp, \
         tc.tile_pool(name="gpool", bufs=2) as gp, \
         tc.tile_pool(name="wpool", bufs=1) as wp, \
         tc.tile_pool(name="opool", bufs=2) as op, \
         tc.tile_pool(name="psr", bufs=6, space="PSUM") as psr, \
         tc.tile_pool(name="pso", bufs=2, space="PSUM") as pso:
        for h in range(NHL):
            v4cm = sp.tile([128, TOK], bf16, tag="v4cm")
            for c, dlt in enumerate((0, 1, 96, 97)):
                if c % 2 == 0:
                    nc.vector.tensor_copy(
                        out=v4cm[32 * c:32 * c + 32, :],
                        in_=v_cm[32 * h:32 * h + 32, dlt:dlt + TOK])
                else:
                    nc.scalar.activation(
                        out=v4cm[32 * c:32 * c + 32, :],
                        in_=v_cm[32 * h:32 * h + 32, dlt:dlt + TOK],
                        func=Act.Copy, scale=1.0)
            tokens = sp.tile([128, TOKR, 128], bf16, tag="tokens")
            nc.sync.dma_start_transpose(out=tokens[:], in_=v4cm[:])

            for half in range(2):
                gt = [gp.tile([128, 1, HALF], bf16, tag=f"g{p}",
                              name=f"gt{p}") for p in range(NP)]
                wexp = [wp.tile([128, HALF], bf16, tag=f"w{p % 2}",
                                name=f"wexp{p}") for p in range(NP)]
                for p in range(NP):
                    hp = h * 4 + p
                    # one 3D-AP broadcast DMA: rows {16c+hp} x32 replication
                    row0 = W4flat[hp:hp + 1, HALF * half:HALF * (half + 1)]
                    src = bass.AP(tensor=row0.tensor, offset=row0.offset,
                                  ap=[[0, 1], [16 * HW, 4], [0, 32],
                                      [1, HALF]])
                    nc.scalar.dma_start(
                        out=wexp[p][:].rearrange("p (a b) -> p a b", a=1),
                        in_=src)
                    nc.gpsimd.dma_gather(
                        gt[p][:], tokens[:],
                        idx_wrap[:, 576 * hp + 288 * half:
                                 576 * hp + 288 * (half + 1)],
                        HALF, HALF, 128, transpose=True,
                        sbuf_tokens_per_rank=128,
                        sbuf_free_dim_per_rank=256,
                        single_packet=False)
                for p in range(NP):
                    nc.vector.tensor_tensor(out=gt[p][:, 0, :],
                                            in0=gt[p][:, 0, :], in1=wexp[p],
                                            op=Alu.mult)
                for n in range(NCH):
                    ng = NCH * half + n
                    bank = psr.tile([32, 384], f32, tag="red")
                    for p in range(NP):
                        nc.tensor.matmul(
                            out=bank, lhsT=redsel,
                            rhs=gt[p][:, 0, 384 * n:384 * (n + 1)],
                            start=(p == 0), stop=(p == 3))
                    nc.scalar.activation(
                        out=sampled[32 * h:32 * h + 32,
                                    384 * ng:384 * (ng + 1)],
                        in_=bank, func=Act.Copy, scale=1.0)

        # out-projection (overlaps the gather tail via Tile deps)
        for n in range(NC384):
            sl = slice(384 * n, 384 * (n + 1))
            for half in range(2):
                ob = pso.tile([128, 384], f32, tag="ob")
                nc.tensor.matmul(out=ob, lhsT=w2[:, half, :],
                                 rhs=sampled[:, sl], start=True, stop=True)
                osb = op.tile([128, 384], f32, tag="osb")
                nc.vector.tensor_copy(out=osb, in_=ob)
                nc.sync.dma_start(
                    out=out_d[half * 128:(half + 1) * 128, sl], in_=osb)


def _host_inputs(inputs):
    x = np.asarray(inputs["x"], dtype=np.float32)
    kv_w = np.asarray(inputs["kv_w"], dtype=np.float32)
    kv_b = np.asarray(inputs["kv_b"], dtype=np.float32)
    off_w = np.asarray(inputs["off_w"], dtype=np.float32)
    off_b = np.asarray(inputs["off_b"], dtype=np.float32)
    aw_w = np.asarray(inputs["aw_w"], dtype=np.float32)
    aw_b = np.asarray(inputs["aw_b"], dtype=np.float32)
    out_w = np.asarray(inputs["out_w"], dtype=np.float32)

    sx = (W - 1.0) / W
    sy = (H - 1.0) / H

    redsel = np.zeros((128, 32), np.float32)
    for c in range(4):
        redsel[32 * c + np.arange(32), np.arange(32)] = 1.0
    sumsel = np.zeros((64, 16), np.float32)
    bcastsel = np.zeros((16, 64), np.float32)
    for q in range(4):
        for hh in range(4):
            for p in range(4):
                sumsel[16 * q + 4 * hh + p, 4 * q + hh] = 1.0
                bcastsel[4 * q + hh, 16 * q + 4 * hh + p] = 1.0

    basein = np.zeros((2, HW), np.float32)
    basein[0] = np.arange(HW) % W
    basein[1] = np.arange(HW) // W
    basew = np.zeros((2, 48), np.float32)
    basew[0, 0:16] = 1.0
    basew[1, 16:32] = 1.0

    bf = ml_dtypes.bfloat16
    in_maps = []
    for core in range(NCORES):
        b, hg = core // 2, core % 2
        heads = list(range(4 * hg, 4 * hg + 4))

        convw = np.zeros((128, 2, 9, 48), np.float32)
        cbias = np.zeros((48, 1), np.float32)
        for j, gh in enumerate(heads):
            for p in range(NP):
                hp = j * 4 + p
                wx = off_w[gh * 8 + p * 2 + 0] * sx
                wy = off_w[gh * 8 + p * 2 + 1] * sy
                wa = aw_w[gh * 4 + p]
                for t in range(9):
                    ky, kx = t // 3, t % 3
                    for cc in range(2):
                        csl = slice(cc * 128, (cc + 1) * 128)
                        convw[:, cc, t, hp] = wx[csl, ky, kx]
                        convw[:, cc, t, 16 + hp] = wy[csl, ky, kx]
                        convw[:, cc, t, 32 + hp] = wa[csl, ky, kx]
                cbias[hp, 0] = off_b[gh * 8 + p * 2 + 0] * sx
                cbias[16 + hp, 0] = off_b[gh * 8 + p * 2 + 1] * sy
                cbias[32 + hp, 0] = aw_b[gh * 4 + p]

        vw = np.zeros((128, 2, 128), np.float32)
        vrows = kv_w[DIM + hg * 128:DIM + (hg + 1) * 128, :]
        for cc in range(2):
            vw[:, cc, :] = vrows[:, cc * 128:(cc + 1) * 128].T
        vb = kv_b[DIM + hg * 128:DIM + (hg + 1) * 128].reshape(128, 1)

        w2 = np.zeros((128, 2, 128), np.float32)
        for halfi in range(2):
            w2[:, halfi, :] = out_w[halfi * 128:(halfi + 1) * 128,
                                    hg * 128:(hg + 1) * 128].T

        in_maps.append({
            "x": np.ascontiguousarray(x[b]),
            "convw": convw.astype(bf),
            "cbias": cbias,
            "basew": basew.astype(bf),
            "basein": basein.astype(bf),
            "vw": vw.astype(bf),
            "vb": np.ascontiguousarray(vb),
            "w2": w2.astype(bf),
            "redsel": redsel.astype(bf),
            "sumsel": sumsel,
            "bcastsel": bcastsel,
        })
    return in_maps


def kernel(**inputs):
    from concourse import bass_utils

    if "nc" not in _CACHE:
        _CACHE["nc"] = _build_nc()
    nc = _CACHE["nc"]

    in_maps = _host_inputs(inputs)
    res = bass_utils.run_bass_kernel_spmd(nc, in_maps,
                                          core_ids=list(range(NCORES)))
    out_b = np.asarray(inputs["out_b"], dtype=np.float32)
    out = np.zeros((4, DIM, HW), np.float32)
    for b in range(4):
        out[b] = (res.results[2 * b]["out"] + res.results[2 * b + 1]["out"]
                  + out_b[:, None])
    return out
